# revision 34
# baseline (speedup 1.0000x reference)
"""GPT decoder (B=8,T=256,D=512,H=8,L=6,DFF=2048,V=50257) on 8 TRN2 NeuronCores.

Strategy (v2):
- Layers data-parallel over batch (core c owns batch c). bf16 matmuls, fp32
  residual/LN/softmax stats.
- All LN gains/biases folded into adjacent weights on the host:
    ln1 -> Wq/Wk/Wv (+ per-feature bias added in the PSUM->SBUF copy),
    ln2 -> W1/b1, lnf -> Wout/bout. FFN b2 added via a K=1 ones-row matmul.
- Attention computes transposed scores S^T[k,q] directly, exp on scalar
  engine, causal mask as a 0/1 multiply on the two diagonal blocks only,
  softmax denominator via ones-vector matmul, normalization applied to the
  attention OUTPUT (folded into the PSUM->SBUF copy). No PE transposes and
  no renormalization pass over the attention matrix.
- Vocab projection tensor-parallel over vocab (6400 cols/core), computed
  vocab-on-partitions: out^T[v,t] so bout fuses into scalar-engine copies.
  Logits stored transposed; host untransposes. AllGather latency hidden by
  a warm-up pass over the core's own tokens read straight from SBUF.
"""
import math
import os

import numpy as np
import ml_dtypes

import concourse.bass as bass
import concourse.tile as tile
from concourse import bacc, mybir
from concourse import bass_utils
from concourse.masks import make_identity

F32 = mybir.dt.float32
BF16 = mybir.dt.bfloat16
I32 = mybir.dt.int32

D = 512
T = 256
H = 8
DK = 64
L = 6
DFF = 2048
V = 50257
B = 8
NCORES = 8
P = 128

VS = 6400           # per-core vocab shard (50 tiles of 128; 8*6400 = 51200 >= V)
NV = VS // P        # 50 vocab row-tiles per core
TT = 2              # token tiles per core (T / P)
KB = D // P         # 4 contraction chunks over D
FB = DFF // P       # 16 chunks over DFF
BT = B * T          # 2048 gathered tokens
WCHUNK = 512        # vocab weight streaming tile (cols)
NWT = VS // WCHUNK  # 12.5 -> handled via list
WTILES = [WCHUNK] * (VS // WCHUNK) + ([VS % WCHUNK] if VS % WCHUNK else [])

_CACHE: dict = {}





def _build_program():
    nc = bacc.Bacc("TRN2", target_bir_lowering=False, debug=False,
                   num_devices=NCORES)

    # ---- I/O declarations ------------------------------------------------
    idx_h = nc.dram_tensor("idx", [T, 1], I32, kind="ExternalInput")
    emb_h = nc.dram_tensor("emb", [V, D], BF16, kind="ExternalInput")
    posenc_h = nc.dram_tensor("posenc", [T, D], F32, kind="ExternalInput")
    tri_h = nc.dram_tensor("tri01", [P, P], BF16, kind="ExternalInput")
    # weights pre-tiled on host: partition dim second-to-innermost
    wqkv_h = nc.dram_tensor("wqkv", [L, 3, P, KB, D], BF16, kind="ExternalInput")
    qkb_h = nc.dram_tensor("qkb", [L, P, 2, KB], F32, kind="ExternalInput")
    vb_h = nc.dram_tensor("vb", [L, D], F32, kind="ExternalInput")
    wo_h = nc.dram_tensor("wo", [L, P, KB, D], BF16, kind="ExternalInput")
    w1_h = nc.dram_tensor("w1", [L, P, KB, DFF], BF16, kind="ExternalInput")
    b1t_h = nc.dram_tensor("b1t", [L, P, FB], F32, kind="ExternalInput")
    w2_h = nc.dram_tensor("w2", [L, P, FB, D], BF16, kind="ExternalInput")
    b2r_h = nc.dram_tensor("b2r", [L, 1, D], BF16, kind="ExternalInput")
    wout_h = nc.dram_tensor("wout", [P, KB, VS], BF16, kind="ExternalInput")
    bout_h = nc.dram_tensor("bout", [P, NV], F32, kind="ExternalInput")
    # logits transposed: [vocab shard, gathered tokens] + warmup dump columns
    logits_h = nc.dram_tensor("logits", [VS, BT + T], F32, kind="ExternalOutput")

    scale = 1.0 / math.sqrt(D)

    def bcast_row(dram_1d_ap, n):
        """DMA-broadcast a [n] DRAM row across all 128 partitions."""
        return bass.AP(tensor=dram_1d_ap.tensor, offset=dram_1d_ap.offset,
                       ap=[[0, P], [1, n]])

    with tile.TileContext(nc) as tc:
        from contextlib import ExitStack
        with ExitStack() as ctx:
            consts = ctx.enter_context(tc.tile_pool(name="consts", bufs=1))
            acts = ctx.enter_context(tc.tile_pool(name="acts", bufs=1))
            scr = ctx.enter_context(tc.tile_pool(name="scr", bufs=3))
            esp = ctx.enter_context(tc.tile_pool(name="esp", bufs=3))
            dram = ctx.enter_context(tc.tile_pool(name="dram", bufs=1, space="DRAM"))

            # ---- constants ----
            ident = consts.tile([P, P], BF16)
            make_identity(nc, ident)
            eps_sb = consts.tile([P, 1], F32)
            nc.vector.memset(eps_sb, 1e-5)
            ones_mat = consts.tile([P, P], BF16)
            nc.vector.memset(ones_mat, 1.0)
            ones_row = consts.tile([1, P], BF16)
            nc.vector.memset(ones_row, 1.0)
            posenc_sb = consts.tile([P, TT, D], F32)
            nc.sync.dma_start(out=posenc_sb,
                              in_=posenc_h.ap().rearrange("(t p) d -> p t d", p=P))
            tri_sb = consts.tile([P, P], BF16)
            nc.sync.dma_start(out=tri_sb, in_=tri_h.ap())

            # ---- persistent activations ----
            x = acts.tile([P, TT, D], F32)          # residual stream
            xn = acts.tile([P, TT, D], BF16)        # post-LN activations
            xnT = acts.tile([P, KB, T], BF16)       # transposed post-LN
            qt = acts.tile([P, KB, T], BF16)        # Q^T (head-pair partitions)
            kt = acts.tile([P, KB, T], BF16)        # K^T
            vv = acts.tile([P, TT, D], BF16)        # V natural [t, h*dk]
            ot = acts.tile([P, KB, T], BF16)        # attn out^T
            ht = acts.tile([P, FB, T], BF16)        # FFN hidden^T

            # ---- embedding gather + positional encoding ----
            idx_sb = acts.tile([P, TT], I32)
            nc.sync.dma_start(out=idx_sb,
                              in_=idx_h.ap().rearrange("(t p) one -> p (t one)", p=P))
            for t in range(TT):
                emb_g = scr.tile([P, D], BF16, name="emb_g")
                nc.gpsimd.indirect_dma_start(
                    out=emb_g[:], out_offset=None,
                    in_=emb_h.ap(),
                    in_offset=bass.IndirectOffsetOnAxis(ap=idx_sb[:, t:t + 1], axis=0),
                )
                nc.vector.tensor_add(out=x[:, t], in0=emb_g, in1=posenc_sb[:, t])

            with tc.tile_pool(name="psB", bufs=2, space="PSUM") as psB, \
                 tc.tile_pool(name="psM", bufs=3, space="PSUM") as psM, \
                 tc.tile_pool(name="psO", bufs=1, space="PSUM") as psO, \
                 tc.tile_pool(name="psT", bufs=2, space="PSUM") as psT:

                def rsqrt_vec(out_f32, v):
                    """out = 1/sqrt(v), vector engine only (no ACT tables):
                    Quake initial guess + two Newton iterations."""
                    ti = scr.tile([P, 1], I32, name="rs_i")
                    nc.vector.tensor_scalar(out=ti, in0=v.bitcast(I32),
                                            scalar1=1, scalar2=0,
                                            op0=mybir.AluOpType.logical_shift_right,
                                            op1=mybir.AluOpType.logical_shift_right)
                    nc.vector.tensor_scalar(out=ti, in0=ti,
                                            scalar1=-1, scalar2=0x5F3759DF,
                                            op0=mybir.AluOpType.mult,
                                            op1=mybir.AluOpType.add)
                    y = out_f32
                    nc.vector.tensor_copy(out=y, in_=ti.bitcast(F32))
                    u = scr.tile([P, 1], F32, name="rs_u")
                    for _ in range(1):
                        nc.vector.tensor_mul(out=u, in0=y, in1=y)
                        nc.vector.tensor_mul(out=u, in0=u, in1=v)
                        nc.vector.tensor_scalar(out=u, in0=u,
                                                scalar1=-0.5, scalar2=1.5,
                                                op0=mybir.AluOpType.mult,
                                                op1=mybir.AluOpType.add)
                        nc.vector.tensor_mul(out=y, in0=y, in1=u)

                def layernorm(out_bf):
                    """Plain LN (no gain/bias: folded into weights downstream)."""
                    for t in range(TT):
                        stats = scr.tile([P, 6], F32, name="ln_stats")
                        nc.vector.bn_stats(out=stats, in_=x[:, t])
                        mv = scr.tile([P, 2], F32, name="ln_mv")
                        nc.vector.bn_aggr(out=mv, in_=stats)
                        vpe = scr.tile([P, 1], F32, name="ln_vpe")
                        nc.vector.tensor_scalar_add(out=vpe, in0=mv[:, 1:2],
                                                    scalar1=1e-5)
                        rstd = scr.tile([P, 1], F32, name="ln_rstd")
                        rsqrt_vec(rstd, vpe)
                        # per-kb chunks: each downstream transpose fires as
                        # soon as its 128-column slice of z is written
                        for kb in range(KB):
                            sl = slice(kb * P, (kb + 1) * P)
                            nc.vector.tensor_scalar(out=out_bf[:, t, sl],
                                                    in0=x[:, t, sl],
                                                    scalar1=mv[:, 0:1],
                                                    scalar2=rstd,
                                                    op0=mybir.AluOpType.subtract,
                                                    op1=mybir.AluOpType.mult)

                def transpose_2x4(src_bf, dst):
                    """[128, TT, D] token-major -> [128, KB, T] feature-major.
                    Both token tiles of one kb pair into a single PSUM tile,
                    drained by one [128, 256] copy."""
                    for kb in range(KB):
                        tp = psT.tile([P, T], BF16, name="pst")
                        nc.tensor.matmul(tp[:, 0:P],
                                         src_bf[:, 0, kb * P:(kb + 1) * P],
                                         ident[:], is_transpose=True,
                                         start=True, stop=False)
                        nc.tensor.matmul(tp[:, P:T],
                                         src_bf[:, 1, kb * P:(kb + 1) * P],
                                         ident[:], is_transpose=True,
                                         start=False, stop=True)
                        if kb % 2 == 0:
                            nc.scalar.activation(
                                out=dst[:, kb], in_=tp[:],
                                func=mybir.ActivationFunctionType.Identity)
                        else:
                            nc.vector.tensor_copy(out=dst[:, kb], in_=tp[:])

                # ================= decoder layers =================
                NPRE = 7     # wout tiles preloaded during the last layer
                vw_pre = ctx.enter_context(tc.tile_pool(name="vw_pre", bufs=NPRE))
                wts = []

                def load_wout_tile(pool, wi):
                    wn = WTILES[wi]
                    wt = pool.tile([P, KB, WCHUNK], BF16, name="wout_t")
                    for hkb in range(2):
                        nc.sync.dma_start(
                            out=wt[:, 2 * hkb:2 * hkb + 2, :wn],
                            in_=wout_h.ap()[:, 2 * hkb:2 * hkb + 2,
                                            wi * WCHUNK:wi * WCHUNK + wn])
                    wts.append(wt)

                with tc.tile_pool(name="wpool", bufs=2) as wp:
                    for l in range(L):
                        # ---- stream this layer's weights (split DMAs) ----
                        wqkv_t = wp.tile([P, 3, KB, D], BF16, name="wqkv_t")
                        for m in range(3):
                            for hkb in range(2):
                                nc.sync.dma_start(
                                    out=wqkv_t[:, m, 2 * hkb:2 * hkb + 2],
                                    in_=wqkv_h.ap()[l, m, :, 2 * hkb:2 * hkb + 2])
                        wo_t = wp.tile([P, KB, D], BF16, name="wo_t")
                        for hkb in range(2):
                            nc.sync.dma_start(
                                out=wo_t[:, 2 * hkb:2 * hkb + 2],
                                in_=wo_h.ap()[l, :, 2 * hkb:2 * hkb + 2])
                        w1_t = wp.tile([P, KB, DFF], BF16, name="w1_t")
                        for kb in range(KB):
                            nc.sync.dma_start(out=w1_t[:, kb],
                                              in_=w1_h.ap()[l, :, kb])
                        w2_t = wp.tile([P, FB, D], BF16, name="w2_t")
                        for qf in range(4):
                            nc.sync.dma_start(
                                out=w2_t[:, 4 * qf:4 * qf + 4],
                                in_=w2_h.ap()[l, :, 4 * qf:4 * qf + 4])
                        b1_sb = wp.tile([P, FB], F32, name="b1_sb")
                        nc.sync.dma_start(out=b1_sb, in_=b1t_h.ap()[l])
                        qkb_sb = wp.tile([P, 2, KB], F32, name="qkb_sb")
                        nc.sync.dma_start(out=qkb_sb, in_=qkb_h.ap()[l])
                        vb_sb = wp.tile([P, D], F32, name="vb_sb")
                        nc.sync.dma_start(out=vb_sb, in_=bcast_row(vb_h.ap()[l], D))
                        b2r_sb = wp.tile([1, D], BF16, name="b2r_sb")
                        nc.sync.dma_start(out=b2r_sb, in_=b2r_h.ap()[l])
                        if l == L - 1:
                            for wi in range(NPRE):
                                load_wout_tile(vw_pre, wi)

                        # ---- LN1 + transpose ----
                        layernorm(xn)
                        transpose_2x4(xn, xnT)

                        # ---- Q^T, K^T (head-pair-major) with folded ln1 bias ----
                        for m, dst in ((0, qt), (1, kt)):
                            for pair in range(KB):
                                ps = psM.tile([P, T], F32, name="psm")
                                for kb in range(KB):
                                    nc.tensor.matmul(
                                        ps[:],
                                        wqkv_t[:, m, kb, pair * P:(pair + 1) * P],
                                        xnT[:, kb],
                                        start=(kb == 0), stop=(kb == KB - 1))
                                if pair % 2 == 0:
                                    nc.scalar.activation(
                                        out=dst[:, pair], in_=ps[:],
                                        func=mybir.ActivationFunctionType.Identity,
                                        bias=qkb_sb[:, m, pair:pair + 1], scale=1.0)
                                else:
                                    nc.vector.tensor_scalar_add(
                                        out=dst[:, pair], in0=ps[:],
                                        scalar1=qkb_sb[:, m, pair:pair + 1])
                        # ---- V natural layout with folded ln1 bias ----
                        for t in range(TT):
                            ps = psB.tile([P, D], F32, name="psb")
                            for kb in range(KB):
                                nc.tensor.matmul(ps[:], xnT[:, kb, t * P:(t + 1) * P],
                                                 wqkv_t[:, 2, kb],
                                                 start=(kb == 0), stop=(kb == KB - 1))
                            nc.vector.tensor_add(out=vv[:, t], in0=ps[:], in1=vb_sb)

                        # ---- attention: transposed scores, no transposes ----
                        for pair in range(KB):
                            ot_ps = psO.tile([P, T], F32, name="pso")
                            rden = scr.tile([P, TT, T], F32, name="rden")
                            for sub in range(2):
                                h = pair * 2 + sub
                                off = sub * DK
                                est = esp.tile([P, TT, T], BF16, name="est")
                                # k-tile 0: all queries
                                s_ps = psM.tile([P, T], F32, name="psm")
                                nc.tensor.matmul(
                                    s_ps[:],
                                    kt[off:off + DK, pair, 0:P],
                                    qt[off:off + DK, pair],
                                    start=True, stop=True)
                                nc.scalar.activation(
                                    out=est[:, 0], in_=s_ps,
                                    func=mybir.ActivationFunctionType.Exp,
                                    scale=scale)
                                # k-tile 1: queries 128..255 only (causal)
                                s_ps2 = psM.tile([P, T], F32, name="psm")
                                nc.tensor.matmul(
                                    s_ps2[:, 0:P],
                                    kt[off:off + DK, pair, P:T],
                                    qt[off:off + DK, pair, P:T],
                                    start=True, stop=True)
                                nc.scalar.activation(
                                    out=est[:, 1, P:T], in_=s_ps2[:, 0:P],
                                    func=mybir.ActivationFunctionType.Exp,
                                    scale=scale)
                                # causal 0/1 mask on the two diagonal blocks
                                nc.vector.tensor_mul(out=est[:, 0, 0:P],
                                                     in0=est[:, 0, 0:P], in1=tri_sb)
                                nc.vector.tensor_mul(out=est[:, 1, P:T],
                                                     in0=est[:, 1, P:T], in1=tri_sb)
                                # denominator replicated on all partitions:
                                # ones[128,128]^T @ est
                                den_ps = psM.tile([P, T], F32, name="psm")
                                nc.tensor.matmul(den_ps[:], ones_mat[:],
                                                 est[:, 0], start=True, stop=False)
                                nc.tensor.matmul(den_ps[:, P:T], ones_mat[:],
                                                 est[:, 1, P:T], start=False,
                                                 stop=True)
                                nc.vector.reciprocal_approx_fast(
                                    out=rden[:, sub], in_=den_ps[:])
                                # attention output (unnormalized): V^T per k-tile
                                nc.tensor.matmul(
                                    ot_ps[off:off + DK, :],
                                    vv[:, 0, h * DK:(h + 1) * DK],
                                    est[:, 0], start=True, stop=False)
                                nc.tensor.matmul(
                                    ot_ps[off:off + DK, P:T],
                                    vv[:, 1, h * DK:(h + 1) * DK],
                                    est[:, 1, P:T], start=False, stop=True)
                            # normalize attention output per head
                            nc.vector.tensor_mul(out=ot[0:DK, pair],
                                                 in0=ot_ps[0:DK, :],
                                                 in1=rden[0:DK, 0])
                            nc.vector.tensor_mul(out=ot[DK:P, pair],
                                                 in0=ot_ps[DK:P, :],
                                                 in1=rden[DK:P, 1])

                        # ---- x += O @ Wo ----
                        for tq in range(TT):
                            ps = psB.tile([P, D], F32, name="psb")
                            for kb in range(KB):
                                nc.tensor.matmul(ps[:], ot[:, kb, tq * P:(tq + 1) * P],
                                                 wo_t[:, kb],
                                                 start=(kb == 0), stop=(kb == KB - 1))
                            nc.vector.tensor_add(out=x[:, tq], in0=x[:, tq], in1=ps[:])

                        # ---- LN2 + FFN (ln2/b1 folded; b2 via ones-row MM) ----
                        layernorm(xn)
                        transpose_2x4(xn, xnT)
                        for fc in range(FB):
                            ps = psM.tile([P, T], F32, name="psm")
                            for kb in range(KB):
                                nc.tensor.matmul(ps[:],
                                                 w1_t[:, kb, fc * P:(fc + 1) * P],
                                                 xnT[:, kb],
                                                 start=(kb == 0), stop=(kb == KB - 1))
                            nc.scalar.activation(out=ht[:, fc], in_=ps[:],
                                                 func=mybir.ActivationFunctionType.Relu,
                                                 bias=b1_sb[:, fc:fc + 1], scale=1.0)
                        for tq in range(TT):
                            ps = psB.tile([P, D], F32, name="psb")
                            for fc in range(FB):
                                nc.tensor.matmul(ps[:], ht[:, fc, tq * P:(tq + 1) * P],
                                                 w2_t[:, fc],
                                                 start=(fc == 0), stop=False)
                            nc.tensor.matmul(ps[:], ones_row[0:1, :],
                                             b2r_sb[0:1, :],
                                             start=False, stop=True)
                            nc.vector.tensor_add(out=x[:, tq], in0=x[:, tq], in1=ps[:])

                # ================= final LN (gain/bias folded into Wout) ======
                layernorm(xn)
                transpose_2x4(xn, xnT)

            # ================= all-gather final activations ==================
            ag_in = dram.tile([D, T], BF16)
            ag_out = dram.tile([NCORES * D, T], BF16, addr_space="Shared")
            for kb in range(KB):
                nc.sync.dma_start(out=ag_in[kb * P:(kb + 1) * P, :], in_=xnT[:, kb])
            nc.gpsimd.collective_compute(
                "AllGather", mybir.AluOpType.bypass,
                replica_groups=[list(range(NCORES))],
                ins=[ag_in[:]], outs=[ag_out[:]])

            # ================= vocab projection ==============================
            with tc.tile_pool(name="vw", bufs=len(WTILES) - NPRE) as vw, \
                 tc.tile_pool(name="vg", bufs=1) as vg, \
                 tc.tile_pool(name="vo", bufs=8) as vo, \
                 tc.tile_pool(name="psV", bufs=5, space="PSUM") as psV, \
                 tc.tile_pool(name="psW", bufs=3, space="PSUM") as psW:
                boutc = vg.tile([P, NV], F32)
                nc.sync.dma_start(out=boutc, in_=bout_h.ap())
                # stream the rest of the wout shard (resident; split DMAs)
                for wi in range(NPRE, len(WTILES)):
                    load_wout_tile(vw, wi)

                def wout_block(v):
                    wt = wts[v // 4]
                    sub = v % 4
                    return wt[:, :, sub * P:(sub + 1) * P]

                # ---- gather all cores' tokens into SBUF (issued first so the
                # transfers fire the moment the collective completes) ----
                xg = vg.tile([P, KB, BT], BF16)
                for b in range(B):
                    for kb in range(KB):
                        nc.sync.dma_start(
                            out=xg[:, kb, b * T:(b + 1) * T],
                            in_=ag_out[b * D + kb * P: b * D + (kb + 1) * P, :])

                # ---- warm-up: own tokens from local xnT while gather runs ----
                for v in range(NV):
                    wb = wout_block(v)
                    ps = psW.tile([P, T], F32, name="psw")
                    for kb in range(KB):
                        nc.tensor.matmul(ps[:], wb[:, kb], xnT[:, kb],
                                         start=(kb == 0), stop=(kb == KB - 1))
                    lg = vo.tile([P, T], F32, name="lgw")
                    if v % 2 == 0:
                        nc.scalar.activation(
                            out=lg, in_=ps[:],
                            func=mybir.ActivationFunctionType.Identity,
                            bias=boutc[:, v:v + 1], scale=1.0)
                    else:
                        nc.vector.tensor_scalar_add(out=lg, in0=ps[:],
                                                    scalar1=boutc[:, v:v + 1])
                    # scalar-engine DGE queue: keeps these stores from queuing
                    # behind the collective-gated xg transfers on sync
                    nc.scalar.dma_start(
                        out=logits_h.ap()[v * P:(v + 1) * P, BT:BT + T], in_=lg)

                # ---- main pass: all 2048 tokens; chunk-outer so compute
                # starts as soon as the first gathered batches land ----
                NT4 = BT // 512  # 4 chunks of 512 tokens
                for tc4 in range(NT4):
                    for v in range(NV):
                        wb = wout_block(v)
                        ps = psV.tile([P, 512], F32, name="psv")
                        for kb in range(KB):
                            nc.tensor.matmul(
                                ps[:], wb[:, kb],
                                xg[:, kb, tc4 * 512:(tc4 + 1) * 512],
                                start=(kb == 0), stop=(kb == KB - 1))
                        lg = vo.tile([P, 512], F32, name="lg")
                        if v % 2 == 0:
                            nc.scalar.activation(
                                out=lg, in_=ps[:],
                                func=mybir.ActivationFunctionType.Identity,
                                bias=boutc[:, v:v + 1], scale=1.0)
                        else:
                            nc.vector.tensor_scalar_add(out=lg, in0=ps[:],
                                                        scalar1=boutc[:, v:v + 1])
                        nc.sync.dma_start(
                            out=logits_h.ap()[v * P:(v + 1) * P,
                                              tc4 * 512:(tc4 + 1) * 512],
                            in_=lg)

    nc.compile()
    return nc


def _prep_inputs(inputs):
    """Host-side fold/cast/shard. Returns per-core input maps."""
    f32 = np.float32
    bf16 = ml_dtypes.bfloat16

    idx = np.asarray(inputs["idx"])
    emb = np.asarray(inputs["emb"], f32)

    # positional encoding (input-independent constant)
    pos = np.arange(T, dtype=np.float64)[:, None]
    div = np.exp(np.arange(0, D, 2, dtype=np.float64) * (-math.log(10000.0) / D))
    pe = np.zeros((T, D), f32)
    pe[:, 0::2] = np.sin(pos * div).astype(f32)
    pe[:, 1::2] = np.cos(pos * div).astype(f32)

    # causal 0/1 mask for a diagonal [k,q] block: valid iff k <= q
    kk, qq = np.meshgrid(np.arange(P), np.arange(P), indexing="ij")
    tri01 = (kk <= qq).astype(f32)

    g1 = np.asarray(inputs["ln1_g"], f32)   # [L, D]
    be1 = np.asarray(inputs["ln1_b"], f32)
    g2 = np.asarray(inputs["ln2_g"], f32)
    be2 = np.asarray(inputs["ln2_b"], f32)
    gf = np.asarray(inputs["lnf_g"], f32)   # [D]
    bef = np.asarray(inputs["lnf_b"], f32)

    wq = np.asarray(inputs["Wq"], f32).transpose(0, 2, 1, 3).reshape(L, D, D)
    wk = np.asarray(inputs["Wk"], f32).transpose(0, 2, 1, 3).reshape(L, D, D)
    wv = np.asarray(inputs["Wv"], f32).transpose(0, 2, 1, 3).reshape(L, D, D)
    # fold ln1 gain into weights; bias becomes per-output-feature addend
    bias_q = np.einsum('ld,ldf->lf', be1, wq)   # [L, D]
    bias_k = np.einsum('ld,ldf->lf', be1, wk)
    bias_v = np.einsum('ld,ldf->lf', be1, wv)
    wq = wq * g1[:, :, None]
    wk = wk * g1[:, :, None]
    wv = wv * g1[:, :, None]
    wqkv = np.stack([wq, wk, wv], axis=1)       # [L, 3, D, D]
    wqkv_t = np.ascontiguousarray(
        wqkv.reshape(L, 3, KB, P, D).transpose(0, 1, 3, 2, 4)).astype(bf16)
    # q/k biases laid out [L, P, 2, KB]: partition p of pair j = feature j*128+p
    qkb = np.stack([bias_q, bias_k], axis=1)    # [L, 2, D]
    qkb = np.ascontiguousarray(
        qkb.reshape(L, 2, KB, P).transpose(0, 3, 1, 2)).astype(f32)

    wo_t = np.ascontiguousarray(
        np.asarray(inputs["Wo"], f32).reshape(L, KB, P, D)
        .transpose(0, 2, 1, 3)).astype(bf16)

    w1 = np.asarray(inputs["W1"], f32)          # [L, D, DFF]
    b1 = np.asarray(inputs["b1"], f32) + np.einsum('ld,ldf->lf', be2, w1)
    w1 = w1 * g2[:, :, None]
    w1_t = np.ascontiguousarray(
        w1.reshape(L, KB, P, DFF).transpose(0, 2, 1, 3)).astype(bf16)
    b1t = np.ascontiguousarray(b1.reshape(L, FB, P).transpose(0, 2, 1))

    w2_t = np.ascontiguousarray(
        np.asarray(inputs["W2"], f32).reshape(L, FB, P, D)
        .transpose(0, 2, 1, 3)).astype(bf16)
    b2r = np.asarray(inputs["b2"], f32).reshape(L, 1, D).astype(bf16)

    wout = np.asarray(inputs["Wout"], f32)      # [D, V]
    bout = np.asarray(inputs["bout"], f32) + bef @ wout
    wout = wout * gf[:, None]
    VPAD = VS * NCORES
    wout_pad = np.zeros((D, VPAD), f32)
    wout_pad[:, :V] = wout
    bout_pad = np.zeros((VPAD,), f32)
    bout_pad[:V] = bout

    common = dict(
        emb=emb.astype(bf16), posenc=pe, tri01=tri01.astype(bf16),
        wqkv=wqkv_t, qkb=qkb, vb=bias_v, wo=wo_t,
        w1=w1_t, b1t=b1t, w2=w2_t, b2r=b2r,
    )
    in_maps = []
    for c in range(NCORES):
        m = dict(common)
        m["idx"] = np.ascontiguousarray(idx[c].astype(np.int32).reshape(T, 1))
        ws = wout_pad[:, c * VS:(c + 1) * VS]
        m["wout"] = np.ascontiguousarray(
            ws.reshape(KB, P, VS).transpose(1, 0, 2)).astype(bf16)
        m["bout"] = np.ascontiguousarray(
            bout_pad[c * VS:(c + 1) * VS].reshape(NV, P).T)
        in_maps.append(m)
    return in_maps


def _unshard(results):
    # each core returns [VS, BT + T] (transposed logits + warmup dump)
    f32 = np.float32
    full = np.empty((B * T, V), f32)
    for c in range(NCORES):
        sh = results[c]["logits"][:, :B * T]     # [VS, BT]
        lo = c * VS
        hi = min((c + 1) * VS, V)
        if hi > lo:
            full[:, lo:hi] = sh[:hi - lo].T
    return np.ascontiguousarray(full.reshape(B, T, V))


def kernel(**inputs):
    if "nc" not in _CACHE:
        _CACHE["nc"] = _build_program()
    nc = _CACHE["nc"]
    in_maps = _prep_inputs(inputs)

    if os.environ.get("KERNEL_USE_SIM"):
        from concourse.bass_interp import MultiCoreSim
        sim = MultiCoreSim(nc, num_cores=NCORES,
                           num_workers=int(os.environ.get("KERNEL_SIM_WORKERS", "8")))
        for c in range(NCORES):
            for name, val in in_maps[c].items():
                sim.cores[c].tensor(name)[:] = val
        sim.simulate()
        results = [
            {"logits": np.array(sim.cores[c].tensor("logits"))}
            for c in range(NCORES)
        ]
        return _unshard(results)

    res = bass_utils.run_bass_kernel_spmd(
        nc, in_maps, core_ids=list(range(NCORES)))
    return _unshard(res.results)


# revision 35
# speedup vs baseline: 1.1036x; 1.1036x over previous
"""GPT decoder (B=8,T=256,D=512,H=8,L=6,DFF=2048,V=50257) on 8 TRN2 NeuronCores.

Strategy (v2):
- Layers data-parallel over batch (core c owns batch c). bf16 matmuls, fp32
  residual/LN/softmax stats.
- All LN gains/biases folded into adjacent weights on the host:
    ln1 -> Wq/Wk/Wv (+ per-feature bias added in the PSUM->SBUF copy),
    ln2 -> W1/b1, lnf -> Wout/bout. FFN b2 added via a K=1 ones-row matmul.
- Attention computes transposed scores S^T[k,q] directly, exp on scalar
  engine, causal mask as a 0/1 multiply on the two diagonal blocks only,
  softmax denominator via ones-vector matmul, normalization applied to the
  attention OUTPUT (folded into the PSUM->SBUF copy). No PE transposes and
  no renormalization pass over the attention matrix.
- Vocab projection tensor-parallel over vocab (6400 cols/core), computed
  vocab-on-partitions: out^T[v,t] so bout fuses into scalar-engine copies.
  Logits stored transposed; host untransposes. AllGather latency hidden by
  a warm-up pass over the core's own tokens read straight from SBUF.
"""
import math
import os

import numpy as np
import ml_dtypes

import concourse.bass as bass
import concourse.tile as tile
from concourse import bacc, mybir
from concourse import bass_utils
from concourse.masks import make_identity

F32 = mybir.dt.float32
BF16 = mybir.dt.bfloat16
I32 = mybir.dt.int32

D = 512
T = 256
H = 8
DK = 64
L = 6
DFF = 2048
V = 50257
B = 8
NCORES = 8
P = 128

VS = 6400           # per-core vocab shard (50 tiles of 128; 8*6400 = 51200 >= V)
NV = VS // P        # 50 vocab row-tiles per core
TT = 2              # token tiles per core (T / P)
KB = D // P         # 4 contraction chunks over D
FB = DFF // P       # 16 chunks over DFF
BT = B * T          # 2048 gathered tokens
WCHUNK = 512        # vocab weight streaming tile (cols)
NWT = VS // WCHUNK  # 12.5 -> handled via list
WTILES = [WCHUNK] * (VS // WCHUNK) + ([VS % WCHUNK] if VS % WCHUNK else [])

_CACHE: dict = {}





def _build_program():
    nc = bacc.Bacc("TRN2", target_bir_lowering=False, debug=False,
                   num_devices=NCORES)

    # ---- I/O declarations ------------------------------------------------
    idx_h = nc.dram_tensor("idx", [T, 1], I32, kind="ExternalInput")
    emb_h = nc.dram_tensor("emb", [V, D], BF16, kind="ExternalInput")
    posenc_h = nc.dram_tensor("posenc", [T, D], F32, kind="ExternalInput")
    tri_h = nc.dram_tensor("tri01", [P, P], BF16, kind="ExternalInput")
    # weights pre-tiled on host: partition dim second-to-innermost
    wqkv_h = nc.dram_tensor("wqkv", [L, 3, P, KB, D], BF16, kind="ExternalInput")
    qkb_h = nc.dram_tensor("qkb", [L, P, 2, KB], F32, kind="ExternalInput")
    vb_h = nc.dram_tensor("vb", [L, D], F32, kind="ExternalInput")
    wo_h = nc.dram_tensor("wo", [L, P, KB, D], BF16, kind="ExternalInput")
    w1_h = nc.dram_tensor("w1", [L, P, KB, DFF], BF16, kind="ExternalInput")
    b1t_h = nc.dram_tensor("b1t", [L, P, FB], F32, kind="ExternalInput")
    w2_h = nc.dram_tensor("w2", [L, P, FB, D], BF16, kind="ExternalInput")
    b2r_h = nc.dram_tensor("b2r", [L, 1, D], BF16, kind="ExternalInput")
    wout_h = nc.dram_tensor("wout", [P, KB, VS], BF16, kind="ExternalInput")
    bout_h = nc.dram_tensor("bout", [P, NV], F32, kind="ExternalInput")
    # logits transposed: [vocab shard, gathered tokens] + warmup dump columns
    logits_h = nc.dram_tensor("logits", [VS, BT + T], F32, kind="ExternalOutput")

    scale = 1.0 / math.sqrt(D)

    def bcast_row(dram_1d_ap, n):
        """DMA-broadcast a [n] DRAM row across all 128 partitions."""
        return bass.AP(tensor=dram_1d_ap.tensor, offset=dram_1d_ap.offset,
                       ap=[[0, P], [1, n]])

    with tile.TileContext(nc) as tc:
        from contextlib import ExitStack
        with ExitStack() as ctx:
            consts = ctx.enter_context(tc.tile_pool(name="consts", bufs=1))
            acts = ctx.enter_context(tc.tile_pool(name="acts", bufs=1))
            scr = ctx.enter_context(tc.tile_pool(name="scr", bufs=3))
            esp = ctx.enter_context(tc.tile_pool(name="esp", bufs=3))
            dram = ctx.enter_context(tc.tile_pool(name="dram", bufs=1, space="DRAM"))

            # ---- constants ----
            ident = consts.tile([P, P], BF16)
            make_identity(nc, ident)
            eps_sb = consts.tile([P, 1], F32)
            nc.vector.memset(eps_sb, 1e-5)
            ones_mat = consts.tile([P, P], BF16)
            nc.vector.memset(ones_mat, 1.0)
            ones_row = consts.tile([1, P], BF16)
            nc.vector.memset(ones_row, 1.0)
            posenc_sb = consts.tile([P, TT, D], F32)
            nc.sync.dma_start(out=posenc_sb,
                              in_=posenc_h.ap().rearrange("(t p) d -> p t d", p=P))
            tri_sb = consts.tile([P, P], BF16)
            nc.sync.dma_start(out=tri_sb, in_=tri_h.ap())

            # ---- persistent activations ----
            x = acts.tile([P, TT, D], F32)          # residual stream
            xn = acts.tile([P, TT, D], BF16)        # post-LN activations
            xnT = acts.tile([P, KB, T], BF16)       # transposed post-LN
            qt = acts.tile([P, KB, T], BF16)        # Q^T (head-pair partitions)
            kt = acts.tile([P, KB, T], BF16)        # K^T
            vv = acts.tile([P, TT, D], BF16)        # V natural [t, h*dk]
            ot = acts.tile([P, KB, T], BF16)        # attn out^T
            ht = acts.tile([P, FB, T], BF16)        # FFN hidden^T

            # ---- embedding gather + positional encoding ----
            idx_sb = acts.tile([P, TT], I32)
            nc.sync.dma_start(out=idx_sb,
                              in_=idx_h.ap().rearrange("(t p) one -> p (t one)", p=P))
            for t in range(TT):
                emb_g = scr.tile([P, D], BF16, name="emb_g")
                nc.gpsimd.indirect_dma_start(
                    out=emb_g[:], out_offset=None,
                    in_=emb_h.ap(),
                    in_offset=bass.IndirectOffsetOnAxis(ap=idx_sb[:, t:t + 1], axis=0),
                )
                nc.vector.tensor_add(out=x[:, t], in0=emb_g, in1=posenc_sb[:, t])

            with tc.tile_pool(name="psB", bufs=2, space="PSUM") as psB, \
                 tc.tile_pool(name="psM", bufs=4, space="PSUM") as psM, \
                 tc.tile_pool(name="psO", bufs=1, space="PSUM") as psO, \
                 tc.tile_pool(name="psT", bufs=1, space="PSUM") as psT:

                def rsqrt_vec(out_f32, v):
                    """out = 1/sqrt(v), vector engine only (no ACT tables):
                    Quake initial guess + two Newton iterations."""
                    ti = scr.tile([P, 1], I32, name="rs_i")
                    nc.vector.tensor_scalar(out=ti, in0=v.bitcast(I32),
                                            scalar1=1, scalar2=0,
                                            op0=mybir.AluOpType.logical_shift_right,
                                            op1=mybir.AluOpType.logical_shift_right)
                    nc.vector.tensor_scalar(out=ti, in0=ti,
                                            scalar1=-1, scalar2=0x5F3759DF,
                                            op0=mybir.AluOpType.mult,
                                            op1=mybir.AluOpType.add)
                    y = out_f32
                    nc.vector.tensor_copy(out=y, in_=ti.bitcast(F32))
                    u = scr.tile([P, 1], F32, name="rs_u")
                    for _ in range(1):
                        nc.vector.tensor_mul(out=u, in0=y, in1=y)
                        nc.vector.tensor_mul(out=u, in0=u, in1=v)
                        nc.vector.tensor_scalar(out=u, in0=u,
                                                scalar1=-0.5, scalar2=1.5,
                                                op0=mybir.AluOpType.mult,
                                                op1=mybir.AluOpType.add)
                        nc.vector.tensor_mul(out=y, in0=y, in1=u)

                def layernorm(out_bf):
                    """Plain LN (no gain/bias: folded into weights downstream)."""
                    for t in range(TT):
                        stats = scr.tile([P, 6], F32, name="ln_stats")
                        nc.vector.bn_stats(out=stats, in_=x[:, t])
                        mv = scr.tile([P, 2], F32, name="ln_mv")
                        nc.vector.bn_aggr(out=mv, in_=stats)
                        vpe = scr.tile([P, 1], F32, name="ln_vpe")
                        nc.vector.tensor_scalar_add(out=vpe, in0=mv[:, 1:2],
                                                    scalar1=1e-5)
                        rstd = scr.tile([P, 1], F32, name="ln_rstd")
                        rsqrt_vec(rstd, vpe)
                        # per-kb chunks: each downstream transpose fires as
                        # soon as its 128-column slice of z is written
                        for kb in range(KB):
                            sl = slice(kb * P, (kb + 1) * P)
                            nc.vector.tensor_scalar(out=out_bf[:, t, sl],
                                                    in0=x[:, t, sl],
                                                    scalar1=mv[:, 0:1],
                                                    scalar2=rstd,
                                                    op0=mybir.AluOpType.subtract,
                                                    op1=mybir.AluOpType.mult)

                def transpose_2x4(src_bf, dst):
                    """[128, TT, D] token-major -> [128, KB, T] feature-major.
                    Both token tiles of one kb pair into a single PSUM tile,
                    drained by one [128, 256] copy."""
                    for kb in range(KB):
                        tp = psT.tile([P, T], BF16, name="pst")
                        nc.tensor.matmul(tp[:, 0:P],
                                         src_bf[:, 0, kb * P:(kb + 1) * P],
                                         ident[:], is_transpose=True,
                                         start=True, stop=False)
                        nc.tensor.matmul(tp[:, P:T],
                                         src_bf[:, 1, kb * P:(kb + 1) * P],
                                         ident[:], is_transpose=True,
                                         start=False, stop=True)
                        if kb % 2 == 0:
                            nc.scalar.activation(
                                out=dst[:, kb], in_=tp[:],
                                func=mybir.ActivationFunctionType.Identity)
                        else:
                            nc.vector.tensor_copy(out=dst[:, kb], in_=tp[:])

                # ================= decoder layers =================
                NPRE = 7     # wout tiles preloaded during the last layer
                vw_pre = ctx.enter_context(tc.tile_pool(name="vw_pre", bufs=NPRE))
                wts = []

                def load_wout_tile(pool, wi):
                    wn = WTILES[wi]
                    wt = pool.tile([P, KB, WCHUNK], BF16, name="wout_t")
                    for hkb in range(2):
                        nc.sync.dma_start(
                            out=wt[:, 2 * hkb:2 * hkb + 2, :wn],
                            in_=wout_h.ap()[:, 2 * hkb:2 * hkb + 2,
                                            wi * WCHUNK:wi * WCHUNK + wn])
                    wts.append(wt)

                with tc.tile_pool(name="wpool", bufs=2) as wp:
                    for l in range(L):
                        # ---- stream this layer's weights (split DMAs) ----
                        wqkv_t = wp.tile([P, 3, KB, D], BF16, name="wqkv_t")
                        for m in range(3):
                            for hkb in range(2):
                                nc.sync.dma_start(
                                    out=wqkv_t[:, m, 2 * hkb:2 * hkb + 2],
                                    in_=wqkv_h.ap()[l, m, :, 2 * hkb:2 * hkb + 2])
                        wo_t = wp.tile([P, KB, D], BF16, name="wo_t")
                        for hkb in range(2):
                            nc.sync.dma_start(
                                out=wo_t[:, 2 * hkb:2 * hkb + 2],
                                in_=wo_h.ap()[l, :, 2 * hkb:2 * hkb + 2])
                        w1_t = wp.tile([P, KB, DFF], BF16, name="w1_t")
                        for kb in range(KB):
                            nc.sync.dma_start(out=w1_t[:, kb],
                                              in_=w1_h.ap()[l, :, kb])
                        w2_t = wp.tile([P, FB, D], BF16, name="w2_t")
                        for qf in range(4):
                            nc.sync.dma_start(
                                out=w2_t[:, 4 * qf:4 * qf + 4],
                                in_=w2_h.ap()[l, :, 4 * qf:4 * qf + 4])
                        b1_sb = wp.tile([P, FB], F32, name="b1_sb")
                        nc.sync.dma_start(out=b1_sb, in_=b1t_h.ap()[l])
                        qkb_sb = wp.tile([P, 2, KB], F32, name="qkb_sb")
                        nc.sync.dma_start(out=qkb_sb, in_=qkb_h.ap()[l])
                        vb_sb = wp.tile([P, D], F32, name="vb_sb")
                        nc.sync.dma_start(out=vb_sb, in_=bcast_row(vb_h.ap()[l], D))
                        b2r_sb = wp.tile([1, D], BF16, name="b2r_sb")
                        nc.sync.dma_start(out=b2r_sb, in_=b2r_h.ap()[l])
                        if l == L - 1:
                            for wi in range(NPRE):
                                load_wout_tile(vw_pre, wi)

                        # ---- LN1 + transpose ----
                        layernorm(xn)
                        transpose_2x4(xn, xnT)

                        # ---- Q^T, K^T (head-pair-major) with folded ln1 bias ----
                        for m, dst in ((0, qt), (1, kt)):
                            for pair in range(KB):
                                ps = psM.tile([P, T], F32, name="psm")
                                for kb in range(KB):
                                    nc.tensor.matmul(
                                        ps[:],
                                        wqkv_t[:, m, kb, pair * P:(pair + 1) * P],
                                        xnT[:, kb],
                                        start=(kb == 0), stop=(kb == KB - 1))
                                if pair % 2 == 0:
                                    nc.scalar.activation(
                                        out=dst[:, pair], in_=ps[:],
                                        func=mybir.ActivationFunctionType.Identity,
                                        bias=qkb_sb[:, m, pair:pair + 1], scale=1.0)
                                else:
                                    nc.vector.tensor_scalar_add(
                                        out=dst[:, pair], in0=ps[:],
                                        scalar1=qkb_sb[:, m, pair:pair + 1])
                        # ---- V natural layout with folded ln1 bias ----
                        for t in range(TT):
                            ps = psB.tile([P, D], F32, name="psb")
                            for kb in range(KB):
                                nc.tensor.matmul(ps[:], xnT[:, kb, t * P:(t + 1) * P],
                                                 wqkv_t[:, 2, kb],
                                                 start=(kb == 0), stop=(kb == KB - 1))
                            nc.vector.tensor_add(out=vv[:, t], in0=ps[:], in1=vb_sb)

                        # ---- attention: transposed scores, no transposes ----
                        for pair in range(KB):
                            ot_ps = psO.tile([P, T], F32, name="pso")
                            rden = scr.tile([P, TT, T], F32, name="rden")
                            for sub in range(2):
                                h = pair * 2 + sub
                                off = sub * DK
                                est = esp.tile([P, TT, T], BF16, name="est")
                                # k-tile 0: all queries
                                s_ps = psM.tile([P, T], F32, name="psm")
                                nc.tensor.matmul(
                                    s_ps[:],
                                    kt[off:off + DK, pair, 0:P],
                                    qt[off:off + DK, pair],
                                    start=True, stop=True)
                                nc.scalar.activation(
                                    out=est[:, 0], in_=s_ps,
                                    func=mybir.ActivationFunctionType.Exp,
                                    scale=scale)
                                # k-tile 1: queries 128..255 only (causal)
                                s_ps2 = psM.tile([P, T], F32, name="psm")
                                nc.tensor.matmul(
                                    s_ps2[:, 0:P],
                                    kt[off:off + DK, pair, P:T],
                                    qt[off:off + DK, pair, P:T],
                                    start=True, stop=True)
                                nc.scalar.activation(
                                    out=est[:, 1, P:T], in_=s_ps2[:, 0:P],
                                    func=mybir.ActivationFunctionType.Exp,
                                    scale=scale)
                                # causal 0/1 mask on the two diagonal blocks
                                nc.vector.tensor_mul(out=est[:, 0, 0:P],
                                                     in0=est[:, 0, 0:P], in1=tri_sb)
                                nc.vector.tensor_mul(out=est[:, 1, P:T],
                                                     in0=est[:, 1, P:T], in1=tri_sb)
                                # denominator replicated on all partitions:
                                # ones[128,128]^T @ est
                                den_ps = psM.tile([P, T], F32, name="psm")
                                nc.tensor.matmul(den_ps[:], ones_mat[:],
                                                 est[:, 0], start=True, stop=False)
                                nc.tensor.matmul(den_ps[:, P:T], ones_mat[:],
                                                 est[:, 1, P:T], start=False,
                                                 stop=True)
                                nc.vector.reciprocal_approx_fast(
                                    out=rden[:, sub], in_=den_ps[:])
                                # attention output (unnormalized): V^T per k-tile
                                nc.tensor.matmul(
                                    ot_ps[off:off + DK, :],
                                    vv[:, 0, h * DK:(h + 1) * DK],
                                    est[:, 0], start=True, stop=False)
                                nc.tensor.matmul(
                                    ot_ps[off:off + DK, P:T],
                                    vv[:, 1, h * DK:(h + 1) * DK],
                                    est[:, 1, P:T], start=False, stop=True)
                            # normalize attention output per head
                            nc.vector.tensor_mul(out=ot[0:DK, pair],
                                                 in0=ot_ps[0:DK, :],
                                                 in1=rden[0:DK, 0])
                            nc.vector.tensor_mul(out=ot[DK:P, pair],
                                                 in0=ot_ps[DK:P, :],
                                                 in1=rden[DK:P, 1])

                        # ---- x += O @ Wo ----
                        for tq in range(TT):
                            ps = psB.tile([P, D], F32, name="psb")
                            for kb in range(KB):
                                nc.tensor.matmul(ps[:], ot[:, kb, tq * P:(tq + 1) * P],
                                                 wo_t[:, kb],
                                                 start=(kb == 0), stop=(kb == KB - 1))
                            nc.vector.tensor_add(out=x[:, tq], in0=x[:, tq], in1=ps[:])

                        # ---- LN2 + FFN (ln2/b1 folded; b2 via ones-row MM) ----
                        layernorm(xn)
                        transpose_2x4(xn, xnT)
                        for fc in range(FB):
                            ps = psM.tile([P, T], F32, name="psm")
                            for kb in range(KB):
                                nc.tensor.matmul(ps[:],
                                                 w1_t[:, kb, fc * P:(fc + 1) * P],
                                                 xnT[:, kb],
                                                 start=(kb == 0), stop=(kb == KB - 1))
                            nc.scalar.activation(out=ht[:, fc], in_=ps[:],
                                                 func=mybir.ActivationFunctionType.Relu,
                                                 bias=b1_sb[:, fc:fc + 1], scale=1.0)
                        for tq in range(TT):
                            ps = psB.tile([P, D], F32, name="psb")
                            for fc in range(FB):
                                nc.tensor.matmul(ps[:], ht[:, fc, tq * P:(tq + 1) * P],
                                                 w2_t[:, fc],
                                                 start=(fc == 0), stop=False)
                            nc.tensor.matmul(ps[:], ones_row[0:1, :],
                                             b2r_sb[0:1, :],
                                             start=False, stop=True)
                            nc.vector.tensor_add(out=x[:, tq], in0=x[:, tq], in1=ps[:])

                # ================= final LN (gain/bias folded into Wout) ======
                layernorm(xn)
                transpose_2x4(xn, xnT)

            # ================= all-gather final activations ==================
            ag_in = dram.tile([D, T], BF16)
            ag_out = dram.tile([NCORES * D, T], BF16, addr_space="Shared")
            for kb in range(KB):
                nc.sync.dma_start(out=ag_in[kb * P:(kb + 1) * P, :], in_=xnT[:, kb])
            nc.gpsimd.collective_compute(
                "AllGather", mybir.AluOpType.bypass,
                replica_groups=[list(range(NCORES))],
                ins=[ag_in[:]], outs=[ag_out[:]])

            # ================= vocab projection ==============================
            with tc.tile_pool(name="vw", bufs=len(WTILES) - NPRE) as vw, \
                 tc.tile_pool(name="vg", bufs=1) as vg, \
                 tc.tile_pool(name="vo", bufs=8) as vo, \
                 tc.tile_pool(name="psV", bufs=5, space="PSUM") as psV, \
                 tc.tile_pool(name="psW", bufs=3, space="PSUM") as psW:
                boutc = vg.tile([P, NV], F32)
                nc.sync.dma_start(out=boutc, in_=bout_h.ap())
                # stream the rest of the wout shard (resident; split DMAs)
                for wi in range(NPRE, len(WTILES)):
                    load_wout_tile(vw, wi)

                def wout_block(v):
                    wt = wts[v // 4]
                    sub = v % 4
                    return wt[:, :, sub * P:(sub + 1) * P]

                # ---- gather all cores' tokens into SBUF (issued first so the
                # transfers fire the moment the collective completes) ----
                xg = vg.tile([P, KB, BT], BF16)
                for b in range(B):
                    for kb in range(KB):
                        nc.sync.dma_start(
                            out=xg[:, kb, b * T:(b + 1) * T],
                            in_=ag_out[b * D + kb * P: b * D + (kb + 1) * P, :])

                # ---- warm-up: own tokens from local xnT while gather runs ----
                for v in range(NV):
                    wb = wout_block(v)
                    ps = psW.tile([P, T], F32, name="psw")
                    for kb in range(KB):
                        nc.tensor.matmul(ps[:], wb[:, kb], xnT[:, kb],
                                         start=(kb == 0), stop=(kb == KB - 1))
                    lg = vo.tile([P, T], F32, name="lgw")
                    if v % 2 == 0:
                        nc.scalar.activation(
                            out=lg, in_=ps[:],
                            func=mybir.ActivationFunctionType.Identity,
                            bias=boutc[:, v:v + 1], scale=1.0)
                    else:
                        nc.vector.tensor_scalar_add(out=lg, in0=ps[:],
                                                    scalar1=boutc[:, v:v + 1])
                    # scalar-engine DGE queue: keeps these stores from queuing
                    # behind the collective-gated xg transfers on sync
                    nc.scalar.dma_start(
                        out=logits_h.ap()[v * P:(v + 1) * P, BT:BT + T], in_=lg)

                # ---- main pass: all 2048 tokens; chunk-outer so compute
                # starts as soon as the first gathered batches land ----
                NT4 = BT // 512  # 4 chunks of 512 tokens
                for tc4 in range(NT4):
                    for v in range(NV):
                        wb = wout_block(v)
                        ps = psV.tile([P, 512], F32, name="psv")
                        for kb in range(KB):
                            nc.tensor.matmul(
                                ps[:], wb[:, kb],
                                xg[:, kb, tc4 * 512:(tc4 + 1) * 512],
                                start=(kb == 0), stop=(kb == KB - 1))
                        lg = vo.tile([P, 512], F32, name="lg")
                        if v % 2 == 0:
                            nc.scalar.activation(
                                out=lg, in_=ps[:],
                                func=mybir.ActivationFunctionType.Identity,
                                bias=boutc[:, v:v + 1], scale=1.0)
                        else:
                            nc.vector.tensor_scalar_add(out=lg, in0=ps[:],
                                                        scalar1=boutc[:, v:v + 1])
                        nc.sync.dma_start(
                            out=logits_h.ap()[v * P:(v + 1) * P,
                                              tc4 * 512:(tc4 + 1) * 512],
                            in_=lg)

    nc.compile()
    return nc


def _prep_inputs(inputs):
    """Host-side fold/cast/shard. Returns per-core input maps."""
    f32 = np.float32
    bf16 = ml_dtypes.bfloat16

    idx = np.asarray(inputs["idx"])
    emb = np.asarray(inputs["emb"], f32)

    # positional encoding (input-independent constant)
    pos = np.arange(T, dtype=np.float64)[:, None]
    div = np.exp(np.arange(0, D, 2, dtype=np.float64) * (-math.log(10000.0) / D))
    pe = np.zeros((T, D), f32)
    pe[:, 0::2] = np.sin(pos * div).astype(f32)
    pe[:, 1::2] = np.cos(pos * div).astype(f32)

    # causal 0/1 mask for a diagonal [k,q] block: valid iff k <= q
    kk, qq = np.meshgrid(np.arange(P), np.arange(P), indexing="ij")
    tri01 = (kk <= qq).astype(f32)

    g1 = np.asarray(inputs["ln1_g"], f32)   # [L, D]
    be1 = np.asarray(inputs["ln1_b"], f32)
    g2 = np.asarray(inputs["ln2_g"], f32)
    be2 = np.asarray(inputs["ln2_b"], f32)
    gf = np.asarray(inputs["lnf_g"], f32)   # [D]
    bef = np.asarray(inputs["lnf_b"], f32)

    wq = np.asarray(inputs["Wq"], f32).transpose(0, 2, 1, 3).reshape(L, D, D)
    wk = np.asarray(inputs["Wk"], f32).transpose(0, 2, 1, 3).reshape(L, D, D)
    wv = np.asarray(inputs["Wv"], f32).transpose(0, 2, 1, 3).reshape(L, D, D)
    # fold ln1 gain into weights; bias becomes per-output-feature addend
    bias_q = np.einsum('ld,ldf->lf', be1, wq)   # [L, D]
    bias_k = np.einsum('ld,ldf->lf', be1, wk)
    bias_v = np.einsum('ld,ldf->lf', be1, wv)
    wq = wq * g1[:, :, None]
    wk = wk * g1[:, :, None]
    wv = wv * g1[:, :, None]
    wqkv = np.stack([wq, wk, wv], axis=1)       # [L, 3, D, D]
    wqkv_t = np.ascontiguousarray(
        wqkv.reshape(L, 3, KB, P, D).transpose(0, 1, 3, 2, 4)).astype(bf16)
    # q/k biases laid out [L, P, 2, KB]: partition p of pair j = feature j*128+p
    qkb = np.stack([bias_q, bias_k], axis=1)    # [L, 2, D]
    qkb = np.ascontiguousarray(
        qkb.reshape(L, 2, KB, P).transpose(0, 3, 1, 2)).astype(f32)

    wo_t = np.ascontiguousarray(
        np.asarray(inputs["Wo"], f32).reshape(L, KB, P, D)
        .transpose(0, 2, 1, 3)).astype(bf16)

    w1 = np.asarray(inputs["W1"], f32)          # [L, D, DFF]
    b1 = np.asarray(inputs["b1"], f32) + np.einsum('ld,ldf->lf', be2, w1)
    w1 = w1 * g2[:, :, None]
    w1_t = np.ascontiguousarray(
        w1.reshape(L, KB, P, DFF).transpose(0, 2, 1, 3)).astype(bf16)
    b1t = np.ascontiguousarray(b1.reshape(L, FB, P).transpose(0, 2, 1))

    w2_t = np.ascontiguousarray(
        np.asarray(inputs["W2"], f32).reshape(L, FB, P, D)
        .transpose(0, 2, 1, 3)).astype(bf16)
    b2r = np.asarray(inputs["b2"], f32).reshape(L, 1, D).astype(bf16)

    wout = np.asarray(inputs["Wout"], f32)      # [D, V]
    bout = np.asarray(inputs["bout"], f32) + bef @ wout
    wout = wout * gf[:, None]
    VPAD = VS * NCORES
    wout_pad = np.zeros((D, VPAD), f32)
    wout_pad[:, :V] = wout
    bout_pad = np.zeros((VPAD,), f32)
    bout_pad[:V] = bout

    common = dict(
        emb=emb.astype(bf16), posenc=pe, tri01=tri01.astype(bf16),
        wqkv=wqkv_t, qkb=qkb, vb=bias_v, wo=wo_t,
        w1=w1_t, b1t=b1t, w2=w2_t, b2r=b2r,
    )
    in_maps = []
    for c in range(NCORES):
        m = dict(common)
        m["idx"] = np.ascontiguousarray(idx[c].astype(np.int32).reshape(T, 1))
        ws = wout_pad[:, c * VS:(c + 1) * VS]
        m["wout"] = np.ascontiguousarray(
            ws.reshape(KB, P, VS).transpose(1, 0, 2)).astype(bf16)
        m["bout"] = np.ascontiguousarray(
            bout_pad[c * VS:(c + 1) * VS].reshape(NV, P).T)
        in_maps.append(m)
    return in_maps


def _unshard(results):
    # each core returns [VS, BT + T] (transposed logits + warmup dump)
    f32 = np.float32
    full = np.empty((B * T, V), f32)
    for c in range(NCORES):
        sh = results[c]["logits"][:, :B * T]     # [VS, BT]
        lo = c * VS
        hi = min((c + 1) * VS, V)
        if hi > lo:
            full[:, lo:hi] = sh[:hi - lo].T
    return np.ascontiguousarray(full.reshape(B, T, V))


def kernel(**inputs):
    if "nc" not in _CACHE:
        _CACHE["nc"] = _build_program()
    nc = _CACHE["nc"]
    in_maps = _prep_inputs(inputs)

    if os.environ.get("KERNEL_USE_SIM"):
        from concourse.bass_interp import MultiCoreSim
        sim = MultiCoreSim(nc, num_cores=NCORES,
                           num_workers=int(os.environ.get("KERNEL_SIM_WORKERS", "8")))
        for c in range(NCORES):
            for name, val in in_maps[c].items():
                sim.cores[c].tensor(name)[:] = val
        sim.simulate()
        results = [
            {"logits": np.array(sim.cores[c].tensor("logits"))}
            for c in range(NCORES)
        ]
        return _unshard(results)

    res = bass_utils.run_bass_kernel_spmd(
        nc, in_maps, core_ids=list(range(NCORES)))
    return _unshard(res.results)


# revision 36
# speedup vs baseline: 1.1171x; 1.0122x over previous
"""GPT decoder (B=8,T=256,D=512,H=8,L=6,DFF=2048,V=50257) on 8 TRN2 NeuronCores.

Strategy (v2):
- Layers data-parallel over batch (core c owns batch c). bf16 matmuls, fp32
  residual/LN/softmax stats.
- All LN gains/biases folded into adjacent weights on the host:
    ln1 -> Wq/Wk/Wv (+ per-feature bias added in the PSUM->SBUF copy),
    ln2 -> W1/b1, lnf -> Wout/bout. FFN b2 added via a K=1 ones-row matmul.
- Attention computes transposed scores S^T[k,q] directly, exp on scalar
  engine, causal mask as a 0/1 multiply on the two diagonal blocks only,
  softmax denominator via ones-vector matmul, normalization applied to the
  attention OUTPUT (folded into the PSUM->SBUF copy). No PE transposes and
  no renormalization pass over the attention matrix.
- Vocab projection tensor-parallel over vocab (6400 cols/core), computed
  vocab-on-partitions: out^T[v,t] so bout fuses into scalar-engine copies.
  Logits stored transposed; host untransposes. AllGather latency hidden by
  a warm-up pass over the core's own tokens read straight from SBUF.
"""
import math
import os

import numpy as np
import ml_dtypes

import concourse.bass as bass
import concourse.tile as tile
from concourse import bacc, mybir
from concourse import bass_utils
from concourse.masks import make_identity

F32 = mybir.dt.float32
BF16 = mybir.dt.bfloat16
I32 = mybir.dt.int32

D = 512
T = 256
H = 8
DK = 64
L = 6
DFF = 2048
V = 50257
B = 8
NCORES = 8
P = 128

VS = 6400           # per-core vocab shard (50 tiles of 128; 8*6400 = 51200 >= V)
NV = VS // P        # 50 vocab row-tiles per core
TT = 2              # token tiles per core (T / P)
KB = D // P         # 4 contraction chunks over D
FB = DFF // P       # 16 chunks over DFF
BT = B * T          # 2048 gathered tokens
WCHUNK = 512        # vocab weight streaming tile (cols)
NWT = VS // WCHUNK  # 12.5 -> handled via list
WTILES = [WCHUNK] * (VS // WCHUNK) + ([VS % WCHUNK] if VS % WCHUNK else [])

_CACHE: dict = {}





def _build_program():
    nc = bacc.Bacc("TRN2", target_bir_lowering=False, debug=False,
                   num_devices=NCORES)

    # ---- I/O declarations ------------------------------------------------
    idx_h = nc.dram_tensor("idx", [T, 1], I32, kind="ExternalInput")
    emb_h = nc.dram_tensor("emb", [V, D], BF16, kind="ExternalInput")
    posenc_h = nc.dram_tensor("posenc", [T, D], F32, kind="ExternalInput")
    tri_h = nc.dram_tensor("tri01", [P, P], BF16, kind="ExternalInput")
    # weights pre-tiled on host: partition dim second-to-innermost
    wqkv_h = nc.dram_tensor("wqkv", [L, 3, P, KB, D], BF16, kind="ExternalInput")
    qkb_h = nc.dram_tensor("qkb", [L, P, 2, KB], F32, kind="ExternalInput")
    vb_h = nc.dram_tensor("vb", [L, D], F32, kind="ExternalInput")
    wo_h = nc.dram_tensor("wo", [L, P, KB, D], BF16, kind="ExternalInput")
    w1_h = nc.dram_tensor("w1", [L, P, KB, DFF], BF16, kind="ExternalInput")
    b1t_h = nc.dram_tensor("b1t", [L, P, FB], F32, kind="ExternalInput")
    w2_h = nc.dram_tensor("w2", [L, P, FB, D], BF16, kind="ExternalInput")
    b2r_h = nc.dram_tensor("b2r", [L, 1, D], BF16, kind="ExternalInput")
    wout_h = nc.dram_tensor("wout", [P, KB, VS], BF16, kind="ExternalInput")
    bout_h = nc.dram_tensor("bout", [P, NV], F32, kind="ExternalInput")
    # logits transposed: [vocab shard, gathered tokens] + warmup dump columns
    logits_h = nc.dram_tensor("logits", [VS, BT + T], F32, kind="ExternalOutput")

    scale = 1.0 / math.sqrt(D)

    def bcast_row(dram_1d_ap, n):
        """DMA-broadcast a [n] DRAM row across all 128 partitions."""
        return bass.AP(tensor=dram_1d_ap.tensor, offset=dram_1d_ap.offset,
                       ap=[[0, P], [1, n]])

    with tile.TileContext(nc) as tc:
        from contextlib import ExitStack
        with ExitStack() as ctx:
            consts = ctx.enter_context(tc.tile_pool(name="consts", bufs=1))
            acts = ctx.enter_context(tc.tile_pool(name="acts", bufs=1))
            scr = ctx.enter_context(tc.tile_pool(name="scr", bufs=3))
            esp = ctx.enter_context(tc.tile_pool(name="esp", bufs=3))
            dram = ctx.enter_context(tc.tile_pool(name="dram", bufs=1, space="DRAM"))

            # ---- constants ----
            ident = consts.tile([P, P], BF16)
            make_identity(nc, ident)
            eps_sb = consts.tile([P, 1], F32)
            nc.vector.memset(eps_sb, 1e-5)
            ones_mat = consts.tile([P, P], BF16)
            nc.vector.memset(ones_mat, 1.0)
            ones_row = consts.tile([1, P], BF16)
            nc.vector.memset(ones_row, 1.0)
            posenc_sb = consts.tile([P, TT, D], F32)
            nc.sync.dma_start(out=posenc_sb,
                              in_=posenc_h.ap().rearrange("(t p) d -> p t d", p=P))
            tri_sb = consts.tile([P, P], BF16)
            nc.sync.dma_start(out=tri_sb, in_=tri_h.ap())

            # ---- persistent activations ----
            x = acts.tile([P, TT, D], F32)          # residual stream
            xn = acts.tile([P, TT, D], BF16)        # post-LN activations
            xnT = acts.tile([P, KB, T], BF16)       # transposed post-LN
            qt = acts.tile([P, KB, T], BF16)        # Q^T (head-pair partitions)
            kt = acts.tile([P, KB, T], BF16)        # K^T
            vv = acts.tile([P, TT, D], BF16)        # V natural [t, h*dk]
            ot = acts.tile([P, KB, T], BF16)        # attn out^T
            ht = acts.tile([P, FB, T], BF16)        # FFN hidden^T

            # ---- embedding gather + positional encoding ----
            idx_sb = acts.tile([P, TT], I32)
            nc.sync.dma_start(out=idx_sb,
                              in_=idx_h.ap().rearrange("(t p) one -> p (t one)", p=P))
            for t in range(TT):
                emb_g = scr.tile([P, D], BF16, name="emb_g")
                nc.gpsimd.indirect_dma_start(
                    out=emb_g[:], out_offset=None,
                    in_=emb_h.ap(),
                    in_offset=bass.IndirectOffsetOnAxis(ap=idx_sb[:, t:t + 1], axis=0),
                )
                nc.vector.tensor_add(out=x[:, t], in0=emb_g, in1=posenc_sb[:, t])

            with tc.tile_pool(name="psB", bufs=2, space="PSUM") as psB, \
                 tc.tile_pool(name="psM", bufs=4, space="PSUM") as psM, \
                 tc.tile_pool(name="psO", bufs=1, space="PSUM") as psO, \
                 tc.tile_pool(name="psT", bufs=1, space="PSUM") as psT:

                def rsqrt_vec(out_f32, v):
                    """out = 1/sqrt(v), vector engine only (no ACT tables):
                    Quake initial guess + two Newton iterations."""
                    ti = scr.tile([P, 1], I32, name="rs_i")
                    nc.vector.tensor_scalar(out=ti, in0=v.bitcast(I32),
                                            scalar1=1, scalar2=0,
                                            op0=mybir.AluOpType.logical_shift_right,
                                            op1=mybir.AluOpType.logical_shift_right)
                    nc.vector.tensor_scalar(out=ti, in0=ti,
                                            scalar1=-1, scalar2=0x5F3759DF,
                                            op0=mybir.AluOpType.mult,
                                            op1=mybir.AluOpType.add)
                    y = out_f32
                    nc.vector.tensor_copy(out=y, in_=ti.bitcast(F32))
                    u = scr.tile([P, 1], F32, name="rs_u")
                    for _ in range(1):
                        nc.vector.tensor_mul(out=u, in0=y, in1=y)
                        nc.vector.tensor_mul(out=u, in0=u, in1=v)
                        nc.vector.tensor_scalar(out=u, in0=u,
                                                scalar1=-0.5, scalar2=1.5,
                                                op0=mybir.AluOpType.mult,
                                                op1=mybir.AluOpType.add)
                        nc.vector.tensor_mul(out=y, in0=y, in1=u)

                def layernorm(out_bf):
                    """Plain LN (no gain/bias: folded into weights downstream)."""
                    for t in range(TT):
                        stats = scr.tile([P, 6], F32, name="ln_stats")
                        nc.vector.bn_stats(out=stats, in_=x[:, t])
                        mv = scr.tile([P, 2], F32, name="ln_mv")
                        nc.vector.bn_aggr(out=mv, in_=stats)
                        vpe = scr.tile([P, 1], F32, name="ln_vpe")
                        nc.vector.tensor_scalar_add(out=vpe, in0=mv[:, 1:2],
                                                    scalar1=1e-5)
                        rstd = scr.tile([P, 1], F32, name="ln_rstd")
                        rsqrt_vec(rstd, vpe)
                        nc.vector.tensor_scalar(out=out_bf[:, t], in0=x[:, t],
                                                scalar1=mv[:, 0:1], scalar2=rstd,
                                                op0=mybir.AluOpType.subtract,
                                                op1=mybir.AluOpType.mult)

                def transpose_2x4(src_bf, dst):
                    """[128, TT, D] token-major -> [128, KB, T] feature-major.
                    Both token tiles of one kb pair into a single PSUM tile,
                    drained by one [128, 256] copy."""
                    for kb in range(KB):
                        tp = psT.tile([P, T], BF16, name="pst")
                        nc.tensor.matmul(tp[:, 0:P],
                                         src_bf[:, 0, kb * P:(kb + 1) * P],
                                         ident[:], is_transpose=True,
                                         start=True, stop=False)
                        nc.tensor.matmul(tp[:, P:T],
                                         src_bf[:, 1, kb * P:(kb + 1) * P],
                                         ident[:], is_transpose=True,
                                         start=False, stop=True)
                        if kb % 2 == 0:
                            nc.scalar.activation(
                                out=dst[:, kb], in_=tp[:],
                                func=mybir.ActivationFunctionType.Identity)
                        else:
                            nc.vector.tensor_copy(out=dst[:, kb], in_=tp[:])

                # ================= decoder layers =================
                NPRE = 7     # wout tiles preloaded during the last layer
                vw_pre = ctx.enter_context(tc.tile_pool(name="vw_pre", bufs=NPRE))
                wts = []

                def load_wout_tile(pool, wi):
                    wn = WTILES[wi]
                    wt = pool.tile([P, KB, WCHUNK], BF16, name="wout_t")
                    for hkb in range(2):
                        nc.sync.dma_start(
                            out=wt[:, 2 * hkb:2 * hkb + 2, :wn],
                            in_=wout_h.ap()[:, 2 * hkb:2 * hkb + 2,
                                            wi * WCHUNK:wi * WCHUNK + wn])
                    wts.append(wt)

                with tc.tile_pool(name="wpool", bufs=2) as wp:
                    for l in range(L):
                        # ---- stream this layer's weights (split DMAs) ----
                        wqkv_t = wp.tile([P, 3, KB, D], BF16, name="wqkv_t")
                        for m in range(3):
                            for hkb in range(2):
                                nc.sync.dma_start(
                                    out=wqkv_t[:, m, 2 * hkb:2 * hkb + 2],
                                    in_=wqkv_h.ap()[l, m, :, 2 * hkb:2 * hkb + 2])
                        wo_t = wp.tile([P, KB, D], BF16, name="wo_t")
                        for hkb in range(2):
                            nc.sync.dma_start(
                                out=wo_t[:, 2 * hkb:2 * hkb + 2],
                                in_=wo_h.ap()[l, :, 2 * hkb:2 * hkb + 2])
                        w1_t = wp.tile([P, KB, DFF], BF16, name="w1_t")
                        for kb in range(KB):
                            nc.sync.dma_start(out=w1_t[:, kb],
                                              in_=w1_h.ap()[l, :, kb])
                        w2_t = wp.tile([P, FB, D], BF16, name="w2_t")
                        for qf in range(4):
                            nc.sync.dma_start(
                                out=w2_t[:, 4 * qf:4 * qf + 4],
                                in_=w2_h.ap()[l, :, 4 * qf:4 * qf + 4])
                        b1_sb = wp.tile([P, FB], F32, name="b1_sb")
                        nc.sync.dma_start(out=b1_sb, in_=b1t_h.ap()[l])
                        qkb_sb = wp.tile([P, 2, KB], F32, name="qkb_sb")
                        nc.sync.dma_start(out=qkb_sb, in_=qkb_h.ap()[l])
                        vb_sb = wp.tile([P, D], F32, name="vb_sb")
                        nc.sync.dma_start(out=vb_sb, in_=bcast_row(vb_h.ap()[l], D))
                        b2r_sb = wp.tile([1, D], BF16, name="b2r_sb")
                        nc.sync.dma_start(out=b2r_sb, in_=b2r_h.ap()[l])
                        if l == L - 1:
                            for wi in range(NPRE):
                                load_wout_tile(vw_pre, wi)

                        # ---- LN1 + transpose ----
                        layernorm(xn)
                        transpose_2x4(xn, xnT)

                        # ---- Q^T, K^T (head-pair-major) with folded ln1 bias ----
                        for m, dst in ((0, qt), (1, kt)):
                            for pair in range(KB):
                                ps = psM.tile([P, T], F32, name="psm")
                                for kb in range(KB):
                                    nc.tensor.matmul(
                                        ps[:],
                                        wqkv_t[:, m, kb, pair * P:(pair + 1) * P],
                                        xnT[:, kb],
                                        start=(kb == 0), stop=(kb == KB - 1))
                                if pair % 2 == 0:
                                    nc.scalar.activation(
                                        out=dst[:, pair], in_=ps[:],
                                        func=mybir.ActivationFunctionType.Identity,
                                        bias=qkb_sb[:, m, pair:pair + 1], scale=1.0)
                                else:
                                    nc.vector.tensor_scalar_add(
                                        out=dst[:, pair], in0=ps[:],
                                        scalar1=qkb_sb[:, m, pair:pair + 1])
                        # ---- V natural layout with folded ln1 bias ----
                        for t in range(TT):
                            ps = psB.tile([P, D], F32, name="psb")
                            for kb in range(KB):
                                nc.tensor.matmul(ps[:], xnT[:, kb, t * P:(t + 1) * P],
                                                 wqkv_t[:, 2, kb],
                                                 start=(kb == 0), stop=(kb == KB - 1))
                            nc.vector.tensor_add(out=vv[:, t], in0=ps[:], in1=vb_sb)

                        # ---- attention: transposed scores, no transposes ----
                        for pair in range(KB):
                            ot_ps = psO.tile([P, T], F32, name="pso")
                            rden = scr.tile([P, TT, T], F32, name="rden")
                            for sub in range(2):
                                h = pair * 2 + sub
                                off = sub * DK
                                est = esp.tile([P, TT, T], BF16, name="est")
                                # k-tile 0: all queries
                                s_ps = psM.tile([P, T], F32, name="psm")
                                nc.tensor.matmul(
                                    s_ps[:],
                                    kt[off:off + DK, pair, 0:P],
                                    qt[off:off + DK, pair],
                                    start=True, stop=True)
                                nc.scalar.activation(
                                    out=est[:, 0], in_=s_ps,
                                    func=mybir.ActivationFunctionType.Exp,
                                    scale=scale)
                                # k-tile 1: queries 128..255 only (causal)
                                s_ps2 = psM.tile([P, T], F32, name="psm")
                                nc.tensor.matmul(
                                    s_ps2[:, 0:P],
                                    kt[off:off + DK, pair, P:T],
                                    qt[off:off + DK, pair, P:T],
                                    start=True, stop=True)
                                nc.scalar.activation(
                                    out=est[:, 1, P:T], in_=s_ps2[:, 0:P],
                                    func=mybir.ActivationFunctionType.Exp,
                                    scale=scale)
                                # causal 0/1 mask on the two diagonal blocks
                                nc.vector.tensor_mul(out=est[:, 0, 0:P],
                                                     in0=est[:, 0, 0:P], in1=tri_sb)
                                nc.vector.tensor_mul(out=est[:, 1, P:T],
                                                     in0=est[:, 1, P:T], in1=tri_sb)
                                # denominator replicated on all partitions:
                                # ones[128,128]^T @ est
                                den_ps = psM.tile([P, T], F32, name="psm")
                                nc.tensor.matmul(den_ps[:], ones_mat[:],
                                                 est[:, 0], start=True, stop=False)
                                nc.tensor.matmul(den_ps[:, P:T], ones_mat[:],
                                                 est[:, 1, P:T], start=False,
                                                 stop=True)
                                nc.vector.reciprocal_approx_fast(
                                    out=rden[:, sub], in_=den_ps[:])
                                # attention output (unnormalized): V^T per k-tile
                                nc.tensor.matmul(
                                    ot_ps[off:off + DK, :],
                                    vv[:, 0, h * DK:(h + 1) * DK],
                                    est[:, 0], start=True, stop=False)
                                nc.tensor.matmul(
                                    ot_ps[off:off + DK, P:T],
                                    vv[:, 1, h * DK:(h + 1) * DK],
                                    est[:, 1, P:T], start=False, stop=True)
                            # normalize attention output per head
                            nc.vector.tensor_mul(out=ot[0:DK, pair],
                                                 in0=ot_ps[0:DK, :],
                                                 in1=rden[0:DK, 0])
                            nc.vector.tensor_mul(out=ot[DK:P, pair],
                                                 in0=ot_ps[DK:P, :],
                                                 in1=rden[DK:P, 1])

                        # ---- x += O @ Wo ----
                        for tq in range(TT):
                            ps = psB.tile([P, D], F32, name="psb")
                            for kb in range(KB):
                                nc.tensor.matmul(ps[:], ot[:, kb, tq * P:(tq + 1) * P],
                                                 wo_t[:, kb],
                                                 start=(kb == 0), stop=(kb == KB - 1))
                            nc.vector.tensor_add(out=x[:, tq], in0=x[:, tq], in1=ps[:])

                        # ---- LN2 + FFN (ln2/b1 folded; b2 via ones-row MM) ----
                        layernorm(xn)
                        transpose_2x4(xn, xnT)
                        for fc in range(FB):
                            ps = psM.tile([P, T], F32, name="psm")
                            for kb in range(KB):
                                nc.tensor.matmul(ps[:],
                                                 w1_t[:, kb, fc * P:(fc + 1) * P],
                                                 xnT[:, kb],
                                                 start=(kb == 0), stop=(kb == KB - 1))
                            nc.scalar.activation(out=ht[:, fc], in_=ps[:],
                                                 func=mybir.ActivationFunctionType.Relu,
                                                 bias=b1_sb[:, fc:fc + 1], scale=1.0)
                        for tq in range(TT):
                            ps = psB.tile([P, D], F32, name="psb")
                            for fc in range(FB):
                                nc.tensor.matmul(ps[:], ht[:, fc, tq * P:(tq + 1) * P],
                                                 w2_t[:, fc],
                                                 start=(fc == 0), stop=False)
                            nc.tensor.matmul(ps[:], ones_row[0:1, :],
                                             b2r_sb[0:1, :],
                                             start=False, stop=True)
                            nc.vector.tensor_add(out=x[:, tq], in0=x[:, tq], in1=ps[:])

                # ================= final LN (gain/bias folded into Wout) ======
                layernorm(xn)
                transpose_2x4(xn, xnT)

            # ================= all-gather final activations ==================
            ag_in = dram.tile([D, T], BF16)
            ag_out = dram.tile([NCORES * D, T], BF16, addr_space="Shared")
            for kb in range(KB):
                nc.sync.dma_start(out=ag_in[kb * P:(kb + 1) * P, :], in_=xnT[:, kb])
            nc.gpsimd.collective_compute(
                "AllGather", mybir.AluOpType.bypass,
                replica_groups=[list(range(NCORES))],
                ins=[ag_in[:]], outs=[ag_out[:]])

            # ================= vocab projection ==============================
            with tc.tile_pool(name="vw", bufs=len(WTILES) - NPRE) as vw, \
                 tc.tile_pool(name="vg", bufs=1) as vg, \
                 tc.tile_pool(name="vo", bufs=8) as vo, \
                 tc.tile_pool(name="psV", bufs=5, space="PSUM") as psV, \
                 tc.tile_pool(name="psW", bufs=3, space="PSUM") as psW:
                boutc = vg.tile([P, NV], F32)
                nc.sync.dma_start(out=boutc, in_=bout_h.ap())
                # stream the rest of the wout shard (resident; split DMAs)
                for wi in range(NPRE, len(WTILES)):
                    load_wout_tile(vw, wi)

                def wout_block(v):
                    wt = wts[v // 4]
                    sub = v % 4
                    return wt[:, :, sub * P:(sub + 1) * P]

                # ---- gather all cores' tokens into SBUF (issued first so the
                # transfers fire the moment the collective completes) ----
                xg = vg.tile([P, KB, BT], BF16)
                for b in range(B):
                    for kb in range(KB):
                        nc.sync.dma_start(
                            out=xg[:, kb, b * T:(b + 1) * T],
                            in_=ag_out[b * D + kb * P: b * D + (kb + 1) * P, :])

                # ---- warm-up: own tokens from local xnT while gather runs ----
                for v in range(NV):
                    wb = wout_block(v)
                    ps = psW.tile([P, T], F32, name="psw")
                    for kb in range(KB):
                        nc.tensor.matmul(ps[:], wb[:, kb], xnT[:, kb],
                                         start=(kb == 0), stop=(kb == KB - 1))
                    lg = vo.tile([P, T], F32, name="lgw")
                    if v % 2 == 0:
                        nc.scalar.activation(
                            out=lg, in_=ps[:],
                            func=mybir.ActivationFunctionType.Identity,
                            bias=boutc[:, v:v + 1], scale=1.0)
                    else:
                        nc.vector.tensor_scalar_add(out=lg, in0=ps[:],
                                                    scalar1=boutc[:, v:v + 1])
                    # scalar-engine DGE queue: keeps these stores from queuing
                    # behind the collective-gated xg transfers on sync
                    nc.scalar.dma_start(
                        out=logits_h.ap()[v * P:(v + 1) * P, BT:BT + T], in_=lg)

                # ---- main pass: all 2048 tokens; chunk-outer so compute
                # starts as soon as the first gathered batches land ----
                NT4 = BT // 512  # 4 chunks of 512 tokens
                for tc4 in range(NT4):
                    for v in range(NV):
                        wb = wout_block(v)
                        ps = psV.tile([P, 512], F32, name="psv")
                        for kb in range(KB):
                            nc.tensor.matmul(
                                ps[:], wb[:, kb],
                                xg[:, kb, tc4 * 512:(tc4 + 1) * 512],
                                start=(kb == 0), stop=(kb == KB - 1))
                        lg = vo.tile([P, 512], F32, name="lg")
                        if v % 2 == 0:
                            nc.scalar.activation(
                                out=lg, in_=ps[:],
                                func=mybir.ActivationFunctionType.Identity,
                                bias=boutc[:, v:v + 1], scale=1.0)
                        else:
                            nc.vector.tensor_scalar_add(out=lg, in0=ps[:],
                                                        scalar1=boutc[:, v:v + 1])
                        nc.sync.dma_start(
                            out=logits_h.ap()[v * P:(v + 1) * P,
                                              tc4 * 512:(tc4 + 1) * 512],
                            in_=lg)

    nc.compile()
    return nc


def _prep_inputs(inputs):
    """Host-side fold/cast/shard. Returns per-core input maps."""
    f32 = np.float32
    bf16 = ml_dtypes.bfloat16

    idx = np.asarray(inputs["idx"])
    emb = np.asarray(inputs["emb"], f32)

    # positional encoding (input-independent constant)
    pos = np.arange(T, dtype=np.float64)[:, None]
    div = np.exp(np.arange(0, D, 2, dtype=np.float64) * (-math.log(10000.0) / D))
    pe = np.zeros((T, D), f32)
    pe[:, 0::2] = np.sin(pos * div).astype(f32)
    pe[:, 1::2] = np.cos(pos * div).astype(f32)

    # causal 0/1 mask for a diagonal [k,q] block: valid iff k <= q
    kk, qq = np.meshgrid(np.arange(P), np.arange(P), indexing="ij")
    tri01 = (kk <= qq).astype(f32)

    g1 = np.asarray(inputs["ln1_g"], f32)   # [L, D]
    be1 = np.asarray(inputs["ln1_b"], f32)
    g2 = np.asarray(inputs["ln2_g"], f32)
    be2 = np.asarray(inputs["ln2_b"], f32)
    gf = np.asarray(inputs["lnf_g"], f32)   # [D]
    bef = np.asarray(inputs["lnf_b"], f32)

    wq = np.asarray(inputs["Wq"], f32).transpose(0, 2, 1, 3).reshape(L, D, D)
    wk = np.asarray(inputs["Wk"], f32).transpose(0, 2, 1, 3).reshape(L, D, D)
    wv = np.asarray(inputs["Wv"], f32).transpose(0, 2, 1, 3).reshape(L, D, D)
    # fold ln1 gain into weights; bias becomes per-output-feature addend
    bias_q = np.einsum('ld,ldf->lf', be1, wq)   # [L, D]
    bias_k = np.einsum('ld,ldf->lf', be1, wk)
    bias_v = np.einsum('ld,ldf->lf', be1, wv)
    wq = wq * g1[:, :, None]
    wk = wk * g1[:, :, None]
    wv = wv * g1[:, :, None]
    wqkv = np.stack([wq, wk, wv], axis=1)       # [L, 3, D, D]
    wqkv_t = np.ascontiguousarray(
        wqkv.reshape(L, 3, KB, P, D).transpose(0, 1, 3, 2, 4)).astype(bf16)
    # q/k biases laid out [L, P, 2, KB]: partition p of pair j = feature j*128+p
    qkb = np.stack([bias_q, bias_k], axis=1)    # [L, 2, D]
    qkb = np.ascontiguousarray(
        qkb.reshape(L, 2, KB, P).transpose(0, 3, 1, 2)).astype(f32)

    wo_t = np.ascontiguousarray(
        np.asarray(inputs["Wo"], f32).reshape(L, KB, P, D)
        .transpose(0, 2, 1, 3)).astype(bf16)

    w1 = np.asarray(inputs["W1"], f32)          # [L, D, DFF]
    b1 = np.asarray(inputs["b1"], f32) + np.einsum('ld,ldf->lf', be2, w1)
    w1 = w1 * g2[:, :, None]
    w1_t = np.ascontiguousarray(
        w1.reshape(L, KB, P, DFF).transpose(0, 2, 1, 3)).astype(bf16)
    b1t = np.ascontiguousarray(b1.reshape(L, FB, P).transpose(0, 2, 1))

    w2_t = np.ascontiguousarray(
        np.asarray(inputs["W2"], f32).reshape(L, FB, P, D)
        .transpose(0, 2, 1, 3)).astype(bf16)
    b2r = np.asarray(inputs["b2"], f32).reshape(L, 1, D).astype(bf16)

    wout = np.asarray(inputs["Wout"], f32)      # [D, V]
    bout = np.asarray(inputs["bout"], f32) + bef @ wout
    wout = wout * gf[:, None]
    VPAD = VS * NCORES
    wout_pad = np.zeros((D, VPAD), f32)
    wout_pad[:, :V] = wout
    bout_pad = np.zeros((VPAD,), f32)
    bout_pad[:V] = bout

    common = dict(
        emb=emb.astype(bf16), posenc=pe, tri01=tri01.astype(bf16),
        wqkv=wqkv_t, qkb=qkb, vb=bias_v, wo=wo_t,
        w1=w1_t, b1t=b1t, w2=w2_t, b2r=b2r,
    )
    in_maps = []
    for c in range(NCORES):
        m = dict(common)
        m["idx"] = np.ascontiguousarray(idx[c].astype(np.int32).reshape(T, 1))
        ws = wout_pad[:, c * VS:(c + 1) * VS]
        m["wout"] = np.ascontiguousarray(
            ws.reshape(KB, P, VS).transpose(1, 0, 2)).astype(bf16)
        m["bout"] = np.ascontiguousarray(
            bout_pad[c * VS:(c + 1) * VS].reshape(NV, P).T)
        in_maps.append(m)
    return in_maps


def _unshard(results):
    # each core returns [VS, BT + T] (transposed logits + warmup dump)
    f32 = np.float32
    full = np.empty((B * T, V), f32)
    for c in range(NCORES):
        sh = results[c]["logits"][:, :B * T]     # [VS, BT]
        lo = c * VS
        hi = min((c + 1) * VS, V)
        if hi > lo:
            full[:, lo:hi] = sh[:hi - lo].T
    return np.ascontiguousarray(full.reshape(B, T, V))


def kernel(**inputs):
    if "nc" not in _CACHE:
        _CACHE["nc"] = _build_program()
    nc = _CACHE["nc"]
    in_maps = _prep_inputs(inputs)

    if os.environ.get("KERNEL_USE_SIM"):
        from concourse.bass_interp import MultiCoreSim
        sim = MultiCoreSim(nc, num_cores=NCORES,
                           num_workers=int(os.environ.get("KERNEL_SIM_WORKERS", "8")))
        for c in range(NCORES):
            for name, val in in_maps[c].items():
                sim.cores[c].tensor(name)[:] = val
        sim.simulate()
        results = [
            {"logits": np.array(sim.cores[c].tensor("logits"))}
            for c in range(NCORES)
        ]
        return _unshard(results)

    res = bass_utils.run_bass_kernel_spmd(
        nc, in_maps, core_ids=list(range(NCORES)))
    return _unshard(res.results)
